# revision 19
# baseline (speedup 1.0000x reference)
"""CrossViewTransformer Trainium2 kernel (v3).

Shards batch B=4 x row-halves over 8 NeuronCores (pure data parallel,
one program, per-core data). Per core:
  q = Wq @ cross_ext          (32, 2176)   fp16 hi/lo split MMs (exact-ish)
  k = Wk @ front_x            (32, 4096)
  energy[j,i] = <q_j, k_i>    K=128 fp16 [qh;ql;qh;ql]x[kh;kh;kl;kl] MMs
  argmax: per-chunk DVE reduce_max + eq*iota accum, block combine
  v = Wv @ x_hat              fp16 MM -> ob-interleaved vbuf; single d=2
                              ap_gather per query-half (overlapped w/ loop)
  conv3x3([front_x; T]) * S + front_x   fp16 MMs; front half overlapped
                                        with the energy loop, T half after.

All weights pre-transposed fp16 on the host; activations pre-split fp16
hi/lo on the host. No PE transposes, no gpsimd iota/broadcast.
"""
import sys

sys.path.insert(0, "/opt/trn_rl_repo")
import numpy as np  # noqa: E402
import concourse.bacc as bacc  # noqa: E402
import concourse.mybir as mybir  # noqa: E402
import concourse.tile as tile  # noqa: E402
from concourse import bass_utils  # noqa: E402
from concourse.bass import AP  # noqa: E402

dt = mybir.dt
ALU = mybir.AluOpType
AX = mybir.AxisListType

B, C, H, W = 4, 256, 64, 64
C8 = C // 8            # 32
HWF = H * W            # 4096 keys
RH = H // 2            # 32 out rows per core
EXTR = RH + 2          # 34 ext rows (1 halo/zero row each side)
EXTQ = EXTR * W        # 2176 ext queries
NBLK = EXTQ // 128     # 17 query blocks
OUTP = RH * W          # 2048 out positions
WP = W + 2             # 66 padded width
CATW = EXTR * WP       # 2244 padded cat row-major size
NCHUNK = 4             # energy chunks of 1024 keys
VCOLS = HWF + 4        # v buffer cols (idx HWF -> zero column), 4-aligned
BSPA = 9               # blocks in gather half A (rows 0..17)
QA = BSPA * 128        # 1152 queries in half A

_CACHED = {}


def _build(has_bqk: bool, has_bv: bool):
    key = (has_bqk, has_bv)
    if key in _CACHED:
        return _CACHED[key]
    nc = bacc.Bacc("TRN2", debug=False)

    # fp16 pre-split activations
    cxh_d = nc.dram_tensor("cxh", (2, 128, EXTQ), dt.float16, kind="ExternalInput")
    cxl_d = nc.dram_tensor("cxl", (2, 128, EXTQ), dt.float16, kind="ExternalInput")
    fxh_d = nc.dram_tensor("fxh", (2, 128, HWF), dt.float16, kind="ExternalInput")
    fxl_d = nc.dram_tensor("fxl", (2, 128, HWF), dt.float16, kind="ExternalInput")
    xh16_d = nc.dram_tensor("xh16", (2, 128, HWF), dt.float16, kind="ExternalInput")
    catf_d = nc.dram_tensor("catf", (2, 128, CATW), dt.float16, kind="ExternalInput")
    # fp16 pre-transposed weights
    wqkT_d = nc.dram_tensor("wqkT", (128, 8 * C8), dt.float16, kind="ExternalInput")
    wvT_d = nc.dram_tensor("wvT", (128, 4 * 128), dt.float16, kind="ExternalInput")
    wfT_d = nc.dram_tensor("wfT", (128, 72 * 128), dt.float16, kind="ExternalInput")
    iota_d = nc.dram_tensor("iota16", (128, HWF), dt.int16, kind="ExternalInput")
    bq_d = nc.dram_tensor("bq", (C8, 1), dt.float32, kind="ExternalInput")
    bk_d = nc.dram_tensor("bk", (C8, 1), dt.float32, kind="ExternalInput")
    bv_d = nc.dram_tensor("bv", (128, 2), dt.float32, kind="ExternalInput")
    bf_d = nc.dram_tensor("bf", (128, 2), dt.float32, kind="ExternalInput")
    mask_d = nc.dram_tensor("mask", (128, NBLK), dt.float32, kind="ExternalInput")
    amask_d = nc.dram_tensor("amask", (128, NBLK), dt.float32, kind="ExternalInput")

    out_d = nc.dram_tensor("out", (2, 128, OUTP), dt.float32, kind="ExternalOutput")
    dbg_arg_d = nc.dram_tensor("dbg_arg", (128, NBLK), dt.float32, kind="ExternalOutput")
    dbg_s_d = nc.dram_tensor("dbg_s", (128, NBLK), dt.float32, kind="ExternalOutput")

    with tile.TileContext(nc) as tc:
        _body(nc, tc, locals(), has_bqk, has_bv)
    nc.compile()
    _CACHED[key] = nc
    return nc


def _body(nc, tc, T, has_bqk, has_bv):
    F32, F16, I16 = dt.float32, dt.float16, dt.int16

    with tc.tile_pool(name="dramscr", bufs=1, space="DRAM") as DR, \
         tc.tile_pool(name="persist", bufs=1) as P, \
         tc.tile_pool(name="pse", bufs=3, space="PSUM") as PSE, \
         tc.tile_pool(name="psb", bufs=2, space="PSUM") as PSB:

        # ---------- persistent tiles ----------
        wqkT = P.tile([128, 8 * C8], F16, tag="wqkT")
        wvT = P.tile([128, 4 * 128], F16, tag="wvT")
        wfT = P.tile([128, 72 * 128], F16, tag="wfT")
        qstack = P.tile([128, EXTQ], F16, tag="qstack")
        kstack = P.tile([128, HWF], F16, tag="kstack")
        vbuf01 = P.tile([128, VCOLS, 2], F32, tag="vbuf01")
        iota16 = P.tile([128, HWF], I16, tag="iota16")
        scr = P.tile([128, 1024], F16, tag="scr")
        cats = []
        for i in range(4):
            ct = P.tile([128, CATW], F16, tag=f"cat{i}")
            cats.append(ct)
        convacc = P.tile([128, 8, 512], F32, tag="convacc")  # (ob*4+g)
        s128 = P.tile([128, OUTP], F32, tag="s128")
        SM = P.tile([128, 128], F32, tag="smalls")
        Af = SM[:, 0:NBLK]
        Mg = SM[:, 17:17 + NBLK]
        arg2 = SM[:, 34:34 + NBLK]
        maskt = SM[:, 51:51 + NBLK]
        amaskt = SM[:, 68:68 + NBLK]
        bqs = SM[0:C8, 85:87]
        bvs = SM[:, 87:89]
        bfs = SM[:, 89:91]
        mch = SM[:, 91:95]
        ach = SM[:, 95:99]
        sel = SM[:, 99:103]
        af16 = P.tile([128, NBLK], I16, tag="af16")
        idxw = P.tile([128, EXTQ // 16], I16, tag="idxw")

        # ---------- input DMAs ----------
        nc.sync.dma_start(wqkT[:, :], T["wqkT_d"].ap())

        ACT_cm = tc.tile_pool(name="actpool", bufs=1)
        ACT = ACT_cm.__enter__()
        cxh = ACT.tile([128, 2, EXTQ], F16, tag="cxh")
        cxl = ACT.tile([128, 2, EXTQ], F16, tag="cxl")
        fxh = ACT.tile([128, 2, HWF], F16, tag="fxh")
        fxl = ACT.tile([128, 2, HWF], F16, tag="fxl")
        xh16 = ACT.tile([128, 2, HWF], F16, tag="xh16")
        for cb in range(2):
            nc.sync.dma_start(cxh[:, cb], T["cxh_d"].ap()[cb])
            nc.sync.dma_start(cxl[:, cb], T["cxl_d"].ap()[cb])
        for cb in range(2):
            nc.sync.dma_start(fxh[:, cb], T["fxh_d"].ap()[cb])
            nc.sync.dma_start(fxl[:, cb], T["fxl_d"].ap()[cb])
        nc.sync.dma_start(wvT[:, :], T["wvT_d"].ap())
        for cb in range(2):
            nc.sync.dma_start(xh16[:, cb], T["xh16_d"].ap()[cb])
        nc.sync.dma_start(wfT[:, :], T["wfT_d"].ap())
        for cb in range(2):
            nc.sync.dma_start(cats[cb][:, :], T["catf_d"].ap()[cb])
        nc.sync.dma_start(iota16[:, :], T["iota_d"].ap())
        nc.sync.dma_start(maskt[:, :], T["mask_d"].ap())
        nc.sync.dma_start(amaskt[:, :], T["amask_d"].ap())
        if has_bqk:
            nc.sync.dma_start(bqs[:, 0:1], T["bq_d"].ap())
            nc.sync.dma_start(bqs[:, 1:2], T["bk_d"].ap())
        if has_bv:
            nc.sync.dma_start(bvs[:, :], T["bv_d"].ap())
        nc.sync.dma_start(bfs[:, :], T["bf_d"].ap())

        nc.vector.memset(cats[2][:, :], 0.0)
        nc.vector.memset(cats[3][:, :], 0.0)
        nc.vector.memset(vbuf01[:, HWF:VCOLS, :], 0.0)

        # ---------- q, k (fp16 hi/lo x hi/lo accumulation) ----------
        def qk_chunk(which, xh_t, xl_t, npos, stack, hrows, lrows, ch):
            n0, n1 = ch * 512, min((ch + 1) * 512, npos)
            pq = PSB.tile([C8, 512], F32, tag="ps512")
            first = True
            for cb in range(2):
                for wsplit in range(2):
                    wcol = ((which * 2 + cb) * 2 + wsplit) * C8
                    for xs, xt in ((0, xh_t), (1, xl_t)):
                        nc.tensor.matmul(
                            pq[:, 0:n1 - n0],
                            wqkT[:, wcol:wcol + C8],
                            xt[:, cb, n0:n1],
                            start=first, stop=(cb == 1 and wsplit == 1 and xs == 1))
                        first = False
            hi = stack[hrows[0]:hrows[0] + C8, n0:n1]
            if has_bqk:
                nc.vector.tensor_scalar(
                    out=hi, in0=pq[:, 0:n1 - n0],
                    scalar1=bqs[:, which:which + 1], scalar2=None, op0=ALU.add)
            else:
                nc.scalar.copy(hi, pq[:, 0:n1 - n0])
            nc.vector.scalar_tensor_tensor(
                stack[lrows[0]:lrows[0] + C8, n0:n1], pq[:, 0:n1 - n0],
                bqs[:, which:which + 1] if has_bqk else 0.0, hi,
                op0=ALU.add, op1=ALU.subtract)
            for extra in hrows[1:]:
                nc.vector.tensor_copy(stack[extra:extra + C8, n0:n1], hi)
            for extra in lrows[1:]:
                nc.vector.tensor_copy(stack[extra:extra + C8, n0:n1],
                                      stack[lrows[0]:lrows[0] + C8, n0:n1])

        # qstack rows: [qh, ql, qh, ql] ; kstack rows: [kh, kh, kl, kl]
        def q_chunk(ch):
            qk_chunk(0, cxh, cxl, EXTQ, qstack, (0, 64), (32, 96), ch)

        def k_chunk(ch):
            qk_chunk(1, fxh, fxl, HWF, kstack, (0, 32), (64, 96), ch)

        q_chunk(0)  # queries 0..511 (blocks 0-3)

        # ---------- filler emitters (PE work overlapped with energy DVE) ----------
        def emit_v_chunk(ch):
            for ob in range(2):
                pv = PSB.tile([128, 512], F32, tag="ps512")
                for cb in range(2):
                    nc.tensor.matmul(pv[:, :],
                                     wvT[:, (cb * 2 + ob) * 128:(cb * 2 + ob + 1) * 128],
                                     xh16[:, cb, ch * 512:(ch + 1) * 512],
                                     start=(cb == 0), stop=(cb == 1))
                vdst = vbuf01[:, ch * 512:(ch + 1) * 512, ob:ob + 1]
                vdst = vdst.rearrange("p n one -> p (n one)")
                if has_bv:
                    nc.vector.tensor_scalar(
                        out=vdst, in0=pv[:, :],
                        scalar1=bvs[:, ob:ob + 1], scalar2=None, op0=ALU.add)
                else:
                    nc.scalar.copy(vdst, pv[:, :])

        def emit_conv_half(ob, g, cb4s, pc):
            for cb4 in cb4s:
                for tap in range(9):
                    dy, dx = tap // 3, tap % 3
                    col = ((cb4 * 9 + tap) * 2 + ob) * 128
                    catv = cats[cb4][:, :].rearrange("p (r wp) -> p r wp", wp=WP)
                    rhs = catv[:, g * 8 + dy:g * 8 + dy + 8, dx:dx + W]
                    nc.tensor.matmul(pc[:, :], wfT[:, col:col + 128], rhs,
                                     start=(cb4 == cb4s[0] and tap == 0),
                                     stop=(cb4 == cb4s[-1] and tap == 8))

        def emit_front_group(ob, g):
            pc = PSB.tile([128, 512], F32, tag="ps512")
            emit_conv_half(ob, g, (0, 1), pc)
            nc.scalar.copy(convacc[:, ob * 4 + g, :], pc[:, :])

        # filler schedule: block -> list of emitters
        fillers = {}
        fillers[1] = [lambda: emit_v_chunk(0), lambda: emit_v_chunk(1)]
        fillers[2] = [lambda: emit_v_chunk(2), lambda: emit_v_chunk(3)]
        fillers[3] = [lambda: emit_v_chunk(4), lambda: emit_v_chunk(5)]
        fillers[4] = [lambda: emit_v_chunk(6), lambda: emit_v_chunk(7)]
        for i, (g, ob) in enumerate([(g, ob) for g in range(4) for ob in range(2)]):
            fillers[5 + i] = [lambda ob=ob, g=g: emit_front_group(ob, g)]

        # ---------- gather pieces ----------
        # piece i covers blocks [PB[i], PB[i+1]); conv-T group g gated on piece GP[g]
        PB = [0, 6, 10, 14, NBLK]
        GATE_CAT = [0.106, 0.133, 0.168, 0.190]   # ms, scheduler placement hints
        GATE_CONV = [0.108, 0.135, 0.170, 0.192]
        wraps = []
        for i in range(4):
            wt = DR.tile([(PB[i + 1] - PB[i]) * 128], I16, tag=f"wrap{i}")
            wraps.append(wt)
        srow_t = DR.tile([EXTQ], F32, tag="srowd")
        TGbox = {}

        def emit_arg_relayout(i):
            b0, b1 = PB[i], PB[i + 1]
            nb = b1 - b0
            nc.vector.tensor_tensor(arg2[:, b0:b1], Af[:, b0:b1],
                                    maskt[:, b0:b1], op=ALU.mult)
            nc.vector.tensor_tensor(arg2[:, b0:b1], arg2[:, b0:b1],
                                    amaskt[:, b0:b1], op=ALU.add)
            nc.vector.tensor_copy(af16[:, b0:b1], arg2[:, b0:b1])
            wsrc = wraps[i][:]
            wdst = AP(wsrc.tensor, wsrc.offset, [[1, 8], [nb * 8, 16], [8, nb]])
            nc.sync.dma_start(wdst, af16[:, b0:b1])
            wview = wraps[i][:].rearrange("(p0 s) -> p0 s", p0=16)
            for g in range(8):
                nc.sync.dma_start(idxw[16 * g:16 * (g + 1), b0 * 8:b1 * 8],
                                  wview[:, :])

        def emit_gather(i):
            q0, q1 = PB[i] * 128, PB[i + 1] * 128
            tg = TGbox["tg"]
            nc.gpsimd.ap_gather(tg[:, q0:q1, :], vbuf01[:, :, :],
                                idxw[:, q0 // 16:q1 // 16],
                                channels=128, num_elems=VCOLS, d=2, num_idxs=q1 - q0)

        def emit_catcopy(i):
            q0, q1 = PB[i] * 128, PB[i + 1] * 128
            tg = TGbox["tg"]
            r0, r1 = q0 // W, q1 // W
            with tc.tile_wait_until(GATE_CAT[i]):
                for ob in range(2):
                    src = tg[:, q0:q1, ob:ob + 1].rearrange("p q one -> p (q one)")
                    src = src.rearrange("p (r w) -> p r w", w=W)
                    dst = cats[2 + ob][:, :].rearrange(
                        "p (r wp) -> p r wp", wp=WP)[:, r0:r1, 1:W + 1]
                    nc.scalar.copy(dst, src)

        def emit_convT(g):
            with tc.tile_wait_until(GATE_CONV[g]):
                for ob in range(2):
                    pc = PSB.tile([128, 512], F32, tag="ps512")
                    emit_conv_half(ob, g, (2, 3), pc)
                    nc.scalar.copy(tstage[:, ob * 4 + g, :], pc[:, :])

        # ---------- energy + argmax ----------
        piece = 0
        for b in range(NBLK):
            for c in range(NCHUNK):
                if b == 0:
                    k_chunk(2 * c)
                    k_chunk(2 * c + 1)
                pe = PSE.tile([128, 1024], F32, tag="pe")
                nc.tensor.matmul(pe[:, 0:512], qstack[:, b * 128:(b + 1) * 128],
                                 kstack[:, c * 1024:c * 1024 + 512], start=True, stop=True)
                nc.tensor.matmul(pe[:, 512:1024], qstack[:, b * 128:(b + 1) * 128],
                                 kstack[:, c * 1024 + 512:(c + 1) * 1024], start=True, stop=True)
                nc.vector.tensor_reduce(mch[:, c:c + 1], pe[:, :], axis=AX.X, op=ALU.max)
                nc.vector.scalar_tensor_tensor(
                    scr[:, :], pe[:, :], mch[:, c:c + 1],
                    iota16[:, c * 1024:(c + 1) * 1024],
                    op0=ALU.is_equal, op1=ALU.mult, accum_out=ach[:, c:c + 1])
            nc.vector.tensor_reduce(Mg[:, b:b + 1], mch[:, :], axis=AX.X, op=ALU.max)
            nc.vector.scalar_tensor_tensor(
                sel[:, :], mch[:, :], Mg[:, b:b + 1], ach[:, :],
                op0=ALU.is_equal, op1=ALU.mult, accum_out=Af[:, b:b + 1])
            if b == 0:
                for qc in range(1, 5):
                    q_chunk(qc)
            for f in fillers.get(b, []):
                f()
            if b == PB[1] - 1:
                # cx/fx/xh all consumed (q/k/v MMs emitted by block 4)
                ACT_cm.__exit__(None, None, None)
                TG_cm = tc.tile_pool(name="tgpool", bufs=1)
                TG = TG_cm.__enter__()
                tgtile = TG.tile([128, EXTQ, 2], F32, tag="tg")
                TGbox["tg"] = tgtile
                tstage = TG.tile([128, 8, 512], F32, tag="tstage")
            if b == PB[piece + 1] - 1:
                emit_arg_relayout(piece)
                emit_gather(piece)
                emit_catcopy(piece)
                emit_convT(piece)
                piece += 1

        nc.sync.dma_start(T["dbg_s_d"].ap(), Mg[:, :])
        nc.sync.dma_start(T["dbg_arg_d"].ap(), Af[:, :])

        # S row: srow[q=b*128+p] = Mg[p,b]; then stride-0 broadcast of out cols
        ssrc = srow_t[:]
        sdst = AP(ssrc.tensor, ssrc.offset, [[1, 128], [128, NBLK]])
        nc.sync.dma_start(sdst, Mg[:, :])
        sbc = AP(ssrc.tensor, ssrc.offset + W, [[0, 128], [1, OUTP]])
        nc.sync.dma_start(s128[:, :], sbc)

        # ---------- assembly ----------
        S_cm = tc.tile_pool(name="stream", bufs=2)
        S = S_cm.__enter__()
        for g in range(4):
            for ob in range(2):
                stage = S.tile([128, 512], F32, tag="stage")
                nc.vector.scalar_tensor_tensor(
                    stage[:, :], tstage[:, ob * 4 + g, :], bfs[:, ob:ob + 1],
                    convacc[:, ob * 4 + g, :], op0=ALU.add, op1=ALU.add)
                nc.vector.tensor_tensor(stage[:, :], stage[:, :],
                                        s128[:, g * 512:(g + 1) * 512], op=ALU.mult)
                fcatv = cats[ob][:, :].rearrange("p (r wp) -> p r wp", wp=WP)
                front_mid = fcatv[:, g * 8 + 1:g * 8 + 9, 1:1 + W]
                nc.vector.tensor_tensor(stage[:, :], stage[:, :], front_mid, op=ALU.add)
                nc.sync.dma_start(T["out_d"].ap()[ob][:, g * 512:(g + 1) * 512],
                                  stage[:, :])
        S_cm.__exit__(None, None, None)
        TG_cm.__exit__(None, None, None)


def _prep_shared(inputs):
    """Weight prep shared by all cores: pre-transposed fp16 (+hi/lo for q/k)."""
    f16, f32 = np.float16, np.float32
    Wq, Wk, Wv = inputs["Wq"], inputs["Wk"], inputs["Wv"]
    Wf = inputs["Wf"].reshape(C, 2 * C, 9)

    wqkT = np.zeros((128, 8 * C8), f16)
    for which, Wx in ((0, Wq), (1, Wk)):
        for cb in range(2):
            blk = np.ascontiguousarray(Wx[:, cb * 128:(cb + 1) * 128].T)  # [128, 32] f32
            hi = blk.astype(f16)
            lo = (blk - hi.astype(f32)).astype(f16)
            base = ((which * 2 + cb) * 2) * C8
            wqkT[:, base:base + C8] = hi
            wqkT[:, base + C8:base + 2 * C8] = lo

    wvT = np.zeros((128, 4 * 128), f16)
    for ob in range(2):
        for cb in range(2):
            wvT[:, (cb * 2 + ob) * 128:(cb * 2 + ob + 1) * 128] = \
                Wv[ob * 128:(ob + 1) * 128, cb * 128:(cb + 1) * 128].T.astype(f16)

    wfT = np.zeros((128, 72 * 128), f16)
    for ob in range(2):
        for cb4 in range(4):
            for tap in range(9):
                col = ((cb4 * 9 + tap) * 2 + ob) * 128
                wfT[:, col:col + 128] = \
                    Wf[ob * 128:(ob + 1) * 128, cb4 * 128:(cb4 + 1) * 128, tap].T.astype(f16)

    iota16 = np.broadcast_to(np.arange(HWF, dtype=np.int16), (128, HWF)).copy()

    return {
        "wqkT": wqkT, "wvT": wvT, "wfT": wfT, "iota16": iota16,
        "bq": inputs["bq"].reshape(C8, 1).astype(f32),
        "bk": inputs["bk"].reshape(C8, 1).astype(f32),
        "bv": np.ascontiguousarray(inputs["bv"].reshape(2, 128).T).astype(f32),
        "bf": np.ascontiguousarray(inputs["bf"].reshape(2, 128).T).astype(f32),
    }


def _hilo(x):
    f16, f32 = np.float16, np.float32
    hi = x.astype(f16)
    lo = (x - hi.astype(f32)).astype(f16)
    return hi, lo


def _prep_core_inputs(inputs, shared, core):
    f16, f32 = np.float16, np.float32
    b, half = core // 2, core % 2
    r0 = half * RH

    def ext_rows(x):  # (C,H,W) -> (C,EXTR,W) with zero boundary row
        out = np.zeros((C, EXTR, W), x.dtype)
        lo, hi = r0 - 1, r0 + RH + 1
        slo, dlo = max(lo, 0), max(lo, 0) - lo
        shi = min(hi, H)
        out[:, dlo:dlo + shi - slo] = x[:, slo:shi]
        return out

    fx = inputs["front_x"][b].reshape(2, 128, HWF)
    fxh, fxl = _hilo(fx)
    cxe = ext_rows(inputs["cross_x"][b]).reshape(2, 128, EXTQ)
    cxh, cxl = _hilo(cxe)
    xh16 = inputs["front_x_hat"][b].reshape(2, 128, HWF).astype(f16)
    catf = np.zeros((C, EXTR, WP), f16)
    catf[:, :, 1:W + 1] = ext_rows(inputs["front_x"][b]).astype(f16)
    catf = catf.reshape(2, 128, CATW)

    valid = np.ones((EXTR, W), f32)
    if r0 == 0:
        valid[0] = 0.0
    if r0 + RH == H:
        valid[-1] = 0.0
    vq = valid.reshape(EXTQ)
    mask = np.empty((128, NBLK), f32)
    for blk in range(NBLK):
        mask[:, blk] = vq[blk * 128:(blk + 1) * 128]
    amask = (1.0 - mask) * HWF

    m = {
        "cxh": np.ascontiguousarray(cxh), "cxl": np.ascontiguousarray(cxl),
        "fxh": np.ascontiguousarray(fxh), "fxl": np.ascontiguousarray(fxl),
        "xh16": np.ascontiguousarray(xh16), "catf": np.ascontiguousarray(catf),
        "mask": mask, "amask": amask,
    }
    m.update(shared)
    return m


LAST_RES = None


def kernel(_trace=False, **inputs):
    global LAST_RES
    inputs = {k: np.asarray(v, dtype=np.float32) for k, v in inputs.items()}
    has_bqk = bool(np.any(inputs["bq"]) or np.any(inputs["bk"]))
    has_bv = bool(np.any(inputs["bv"]))
    nc = _build(has_bqk, has_bv)
    shared = _prep_shared(inputs)
    in_maps = [_prep_core_inputs(inputs, shared, core) for core in range(8)]
    kw = {"trace": True} if _trace else {}
    res = bass_utils.run_bass_kernel_spmd(nc, in_maps, core_ids=list(range(8)), **kw)
    LAST_RES = res
    out = np.empty((B, C, H, W), np.float32)
    for core in range(8):
        b, half = core // 2, core % 2
        o = res.results[core]["out"].reshape(C, RH, W)
        out[b, :, half * RH:(half + 1) * RH, :] = o
    return out


if __name__ == "__main__":
    rng = np.random.default_rng(0)
    ins = {
        "front_x": rng.standard_normal((B, C, H, W)).astype(np.float32),
        "cross_x": rng.standard_normal((B, C, H, W)).astype(np.float32),
        "front_x_hat": rng.standard_normal((B, C, H, W)).astype(np.float32),
        "Wq": (rng.standard_normal((C8, C)) / 16).astype(np.float32),
        "bq": np.zeros((C8,), np.float32),
        "Wk": (rng.standard_normal((C8, C)) / 16).astype(np.float32),
        "bk": np.zeros((C8,), np.float32),
        "Wv": (rng.standard_normal((C, C)) / 16).astype(np.float32),
        "bv": np.zeros((C,), np.float32),
        "Wf": (rng.standard_normal((C, 2 * C, 3, 3)) / 68).astype(np.float32),
        "bf": np.zeros((C,), np.float32),
    }
    out = kernel(**ins)
    print("kernel ran, out shape", out.shape, "std", out.std())


# revision 20
# speedup vs baseline: 1.2951x; 1.2951x over previous
"""CrossViewTransformer Trainium2 kernel (v3).

Shards batch B=4 x row-halves over 8 NeuronCores (pure data parallel,
one program, per-core data). Per core:
  q = Wq @ cross_ext          (32, 2176)   fp16 hi/lo split MMs (exact-ish)
  k = Wk @ front_x            (32, 4096)
  energy[j,i] = <q_j, k_i>    K=128 fp16 [qh;ql;qh;ql]x[kh;kh;kl;kl] MMs
  argmax: per-chunk DVE reduce_max + eq*iota accum, block combine
  v = Wv @ x_hat              fp16 MM -> ob-interleaved vbuf; single d=2
                              ap_gather per query-half (overlapped w/ loop)
  conv3x3([front_x; T]) * S + front_x   fp16 MMs; front half overlapped
                                        with the energy loop, T half after.

All weights pre-transposed fp16 on the host; activations pre-split fp16
hi/lo on the host. No PE transposes, no gpsimd iota/broadcast.
"""
import sys

sys.path.insert(0, "/opt/trn_rl_repo")
import numpy as np  # noqa: E402
import concourse.bacc as bacc  # noqa: E402
import concourse.mybir as mybir  # noqa: E402
import concourse.tile as tile  # noqa: E402
from concourse import bass_utils  # noqa: E402
from concourse.bass import AP  # noqa: E402

dt = mybir.dt
ALU = mybir.AluOpType
AX = mybir.AxisListType

B, C, H, W = 4, 256, 64, 64
C8 = C // 8            # 32
HWF = H * W            # 4096 keys
RH = H // 2            # 32 out rows per core
EXTR = RH + 2          # 34 ext rows (1 halo/zero row each side)
EXTQ = EXTR * W        # 2176 ext queries
NBLK = EXTQ // 128     # 17 query blocks
OUTP = RH * W          # 2048 out positions
WP = W + 2             # 66 padded width
CATW = EXTR * WP       # 2244 padded cat row-major size
NCHUNK = 4             # energy chunks of 1024 keys
VCOLS = HWF + 4        # v buffer cols (idx HWF -> zero column), 4-aligned
BSPA = 9               # blocks in gather half A (rows 0..17)
QA = BSPA * 128        # 1152 queries in half A

_CACHED = {}


def _build(has_bqk: bool, has_bv: bool):
    key = (has_bqk, has_bv)
    if key in _CACHED:
        return _CACHED[key]
    nc = bacc.Bacc("TRN2", debug=False)

    # fp16 pre-split activations
    cxh_d = nc.dram_tensor("cxh", (2, 128, EXTQ), dt.float16, kind="ExternalInput")
    cxl_d = nc.dram_tensor("cxl", (2, 128, EXTQ), dt.float16, kind="ExternalInput")
    fxh_d = nc.dram_tensor("fxh", (2, 128, HWF), dt.float16, kind="ExternalInput")
    fxl_d = nc.dram_tensor("fxl", (2, 128, HWF), dt.float16, kind="ExternalInput")
    xh16_d = nc.dram_tensor("xh16", (2, 128, HWF), dt.float16, kind="ExternalInput")
    catf_d = nc.dram_tensor("catf", (2, 128, CATW), dt.float16, kind="ExternalInput")
    # fp16 pre-transposed weights
    wqkT_d = nc.dram_tensor("wqkT", (128, 8 * C8), dt.float16, kind="ExternalInput")
    wvT_d = nc.dram_tensor("wvT", (128, 4 * 128), dt.float16, kind="ExternalInput")
    wfT_d = nc.dram_tensor("wfT", (128, 72 * 128), dt.float16, kind="ExternalInput")
    iota_d = nc.dram_tensor("iota16", (128, HWF), dt.int16, kind="ExternalInput")
    bq_d = nc.dram_tensor("bq", (C8, 1), dt.float32, kind="ExternalInput")
    bk_d = nc.dram_tensor("bk", (C8, 1), dt.float32, kind="ExternalInput")
    bv_d = nc.dram_tensor("bv", (128, 2), dt.float32, kind="ExternalInput")
    bf_d = nc.dram_tensor("bf", (128, 2), dt.float32, kind="ExternalInput")
    mask_d = nc.dram_tensor("mask", (128, NBLK), dt.float32, kind="ExternalInput")
    amask_d = nc.dram_tensor("amask", (128, NBLK), dt.float32, kind="ExternalInput")

    out_d = nc.dram_tensor("out", (2, 128, OUTP), dt.float32, kind="ExternalOutput")
    dbg_arg_d = nc.dram_tensor("dbg_arg", (128, NBLK), dt.float32, kind="ExternalOutput")
    dbg_s_d = nc.dram_tensor("dbg_s", (128, NBLK), dt.float32, kind="ExternalOutput")

    with tile.TileContext(nc) as tc:
        _body(nc, tc, locals(), has_bqk, has_bv)
    nc.compile()
    _CACHED[key] = nc
    return nc


def _body(nc, tc, T, has_bqk, has_bv):
    F32, F16, I16 = dt.float32, dt.float16, dt.int16

    with tc.tile_pool(name="dramscr", bufs=1, space="DRAM") as DR, \
         tc.tile_pool(name="persist", bufs=1) as P, \
         tc.tile_pool(name="pse", bufs=3, space="PSUM") as PSE, \
         tc.tile_pool(name="psb", bufs=2, space="PSUM") as PSB:

        # ---------- persistent tiles ----------
        wqkT = P.tile([128, 8 * C8], F16, tag="wqkT")
        wvT = P.tile([128, 4 * 128], F16, tag="wvT")
        wfT = P.tile([128, 72 * 128], F16, tag="wfT")
        qstack = P.tile([128, EXTQ], F16, tag="qstack")
        kstack = P.tile([128, HWF], F16, tag="kstack")
        vbuf01 = P.tile([128, VCOLS, 2], F32, tag="vbuf01")
        iota16 = P.tile([128, HWF], I16, tag="iota16")
        scr = P.tile([128, 1024], F16, tag="scr")
        cats = []
        for i in range(4):
            ct = P.tile([128, CATW], F16, tag=f"cat{i}")
            cats.append(ct)
        convacc = P.tile([128, 8, 512], F32, tag="convacc")  # (ob*4+g)
        s128 = P.tile([128, OUTP], F32, tag="s128")
        SM = P.tile([128, 128], F32, tag="smalls")
        Af = SM[:, 0:NBLK]
        Mg = SM[:, 17:17 + NBLK]
        arg2 = SM[:, 34:34 + NBLK]
        maskt = SM[:, 51:51 + NBLK]
        amaskt = SM[:, 68:68 + NBLK]
        bqs = SM[0:C8, 85:87]
        bvs = SM[:, 87:89]
        bfs = SM[:, 89:91]
        mch = SM[:, 91:95]
        ach = SM[:, 95:99]
        sel = SM[:, 99:103]
        mkscr = P.tile([128, W], F32, tag="mkscr")
        af16 = P.tile([128, NBLK], I16, tag="af16")
        idxw = P.tile([128, EXTQ // 16], I16, tag="idxw")

        # ---------- input DMAs ----------
        nc.sync.dma_start(wqkT[:, :], T["wqkT_d"].ap())

        ACT_cm = tc.tile_pool(name="actpool", bufs=1)
        ACT = ACT_cm.__enter__()
        cxh = ACT.tile([128, 2, EXTQ], F16, tag="cxh")
        cxl = ACT.tile([128, 2, EXTQ], F16, tag="cxl")
        fxh = ACT.tile([128, 2, HWF], F16, tag="fxh")
        fxl = ACT.tile([128, 2, HWF], F16, tag="fxl")
        xh16 = ACT.tile([128, 2, HWF], F16, tag="xh16")
        for cb in range(2):
            nc.sync.dma_start(cxh[:, cb], T["cxh_d"].ap()[cb])
            nc.sync.dma_start(cxl[:, cb], T["cxl_d"].ap()[cb])
        for cb in range(2):
            nc.sync.dma_start(fxh[:, cb], T["fxh_d"].ap()[cb])
            nc.sync.dma_start(fxl[:, cb], T["fxl_d"].ap()[cb])
        nc.sync.dma_start(wvT[:, :], T["wvT_d"].ap())
        for cb in range(2):
            nc.sync.dma_start(xh16[:, cb], T["xh16_d"].ap()[cb])
        nc.sync.dma_start(wfT[:, :], T["wfT_d"].ap())
        for cb in range(2):
            nc.sync.dma_start(cats[cb][:, :], T["catf_d"].ap()[cb])
        nc.sync.dma_start(iota16[:, :], T["iota_d"].ap())
        nc.sync.dma_start(maskt[:, :], T["mask_d"].ap())
        nc.sync.dma_start(amaskt[:, :], T["amask_d"].ap())
        if has_bqk:
            nc.sync.dma_start(bqs[:, 0:1], T["bq_d"].ap())
            nc.sync.dma_start(bqs[:, 1:2], T["bk_d"].ap())
        if has_bv:
            nc.sync.dma_start(bvs[:, :], T["bv_d"].ap())
        nc.sync.dma_start(bfs[:, :], T["bf_d"].ap())

        nc.vector.memset(cats[2][:, :], 0.0)
        nc.vector.memset(cats[3][:, :], 0.0)
        nc.vector.memset(vbuf01[:, HWF:VCOLS, :], 0.0)

        # ---------- q, k (fp16 hi/lo x hi/lo accumulation) ----------
        def qk_chunk(which, xh_t, xl_t, npos, stack, hrows, lrows, ch):
            n0, n1 = ch * 512, min((ch + 1) * 512, npos)
            pq = PSB.tile([C8, 512], F32, tag="ps512")
            first = True
            for cb in range(2):
                for wsplit in range(2):
                    wcol = ((which * 2 + cb) * 2 + wsplit) * C8
                    for xs, xt in ((0, xh_t), (1, xl_t)):
                        nc.tensor.matmul(
                            pq[:, 0:n1 - n0],
                            wqkT[:, wcol:wcol + C8],
                            xt[:, cb, n0:n1],
                            start=first, stop=(cb == 1 and wsplit == 1 and xs == 1))
                        first = False
            hi = stack[hrows[0]:hrows[0] + C8, n0:n1]
            if has_bqk:
                nc.vector.tensor_scalar(
                    out=hi, in0=pq[:, 0:n1 - n0],
                    scalar1=bqs[:, which:which + 1], scalar2=None, op0=ALU.add)
            else:
                nc.scalar.copy(hi, pq[:, 0:n1 - n0])
            nc.vector.scalar_tensor_tensor(
                stack[lrows[0]:lrows[0] + C8, n0:n1], pq[:, 0:n1 - n0],
                bqs[:, which:which + 1] if has_bqk else 0.0, hi,
                op0=ALU.add, op1=ALU.subtract)
            for extra in hrows[1:]:
                nc.vector.tensor_copy(stack[extra:extra + C8, n0:n1], hi)
            for extra in lrows[1:]:
                nc.vector.tensor_copy(stack[extra:extra + C8, n0:n1],
                                      stack[lrows[0]:lrows[0] + C8, n0:n1])

        # qstack rows: [qh, ql, qh, ql] ; kstack rows: [kh, kh, kl, kl]
        def q_chunk(ch):
            qk_chunk(0, cxh, cxl, EXTQ, qstack, (0, 64), (32, 96), ch)

        def k_chunk(ch):
            qk_chunk(1, fxh, fxl, HWF, kstack, (0, 32), (64, 96), ch)

        q_chunk(0)  # queries 0..511 (blocks 0-3)

        # ---------- filler emitters (PE work overlapped with energy DVE) ----------
        def emit_v_chunk(ch):
            for ob in range(2):
                pv = PSB.tile([128, 512], F32, tag="ps512")
                for cb in range(2):
                    nc.tensor.matmul(pv[:, :],
                                     wvT[:, (cb * 2 + ob) * 128:(cb * 2 + ob + 1) * 128],
                                     xh16[:, cb, ch * 512:(ch + 1) * 512],
                                     start=(cb == 0), stop=(cb == 1))
                vdst = vbuf01[:, ch * 512:(ch + 1) * 512, ob:ob + 1]
                vdst = vdst.rearrange("p n one -> p (n one)")
                if has_bv:
                    nc.vector.tensor_scalar(
                        out=vdst, in0=pv[:, :],
                        scalar1=bvs[:, ob:ob + 1], scalar2=None, op0=ALU.add)
                else:
                    nc.scalar.copy(vdst, pv[:, :])

        def emit_conv_half(ob, g, cb4s, pc):
            for cb4 in cb4s:
                for tap in range(9):
                    dy, dx = tap // 3, tap % 3
                    col = ((cb4 * 9 + tap) * 2 + ob) * 128
                    catv = cats[cb4][:, :].rearrange("p (r wp) -> p r wp", wp=WP)
                    rhs = catv[:, g * 8 + dy:g * 8 + dy + 8, dx:dx + W]
                    nc.tensor.matmul(pc[:, :], wfT[:, col:col + 128], rhs,
                                     start=(cb4 == cb4s[0] and tap == 0),
                                     stop=(cb4 == cb4s[-1] and tap == 8))

        def emit_front_group(ob, g):
            pc = PSB.tile([128, 512], F32, tag="ps512")
            emit_conv_half(ob, g, (0, 1), pc)
            nc.scalar.copy(convacc[:, ob * 4 + g, :], pc[:, :])

        # filler schedule: block -> list of emitters
        fillers = {}
        fillers[1] = [lambda: emit_v_chunk(0), lambda: emit_v_chunk(1)]
        fillers[2] = [lambda: emit_v_chunk(2), lambda: emit_v_chunk(3)]
        fillers[3] = [lambda: emit_v_chunk(4), lambda: emit_v_chunk(5)]
        fillers[4] = [lambda: emit_v_chunk(6), lambda: emit_v_chunk(7)]
        for i, (g, ob) in enumerate([(g, ob) for g in range(4) for ob in range(2)]):
            fillers[5 + i] = [lambda ob=ob, g=g: emit_front_group(ob, g)]

        # ---------- gather pieces ----------
        # piece i covers blocks [PB[i], PB[i+1]); conv-T group g gated on piece GP[g]
        PB = [0, 6, 10, 15, NBLK]
        MARKER_AT = {10: 0, 14: 1}   # block -> piece whose catcopy/convT it gates
        wraps = []
        for i in range(4):
            wt = DR.tile([(PB[i + 1] - PB[i]) * 128], I16, tag=f"wrap{i}")
            wraps.append(wt)
        srow_t = DR.tile([EXTQ], F32, tag="srowd")
        TGbox = {}

        def emit_arg_relayout(i):
            b0, b1 = PB[i], PB[i + 1]
            nb = b1 - b0
            nc.vector.tensor_tensor(arg2[:, b0:b1], Af[:, b0:b1],
                                    maskt[:, b0:b1], op=ALU.mult)
            nc.vector.tensor_tensor(arg2[:, b0:b1], arg2[:, b0:b1],
                                    amaskt[:, b0:b1], op=ALU.add)
            nc.vector.tensor_copy(af16[:, b0:b1], arg2[:, b0:b1])
            wsrc = wraps[i][:]
            wdst = AP(wsrc.tensor, wsrc.offset, [[1, 8], [nb * 8, 16], [8, nb]])
            nc.sync.dma_start(wdst, af16[:, b0:b1])
            wview = wraps[i][:].rearrange("(p0 s) -> p0 s", p0=16)
            for g in range(8):
                nc.sync.dma_start(idxw[16 * g:16 * (g + 1), b0 * 8:b1 * 8],
                                  wview[:, :])

        def emit_gather(i):
            q0, q1 = PB[i] * 128, PB[i + 1] * 128
            tg = TGbox["tg"]
            nc.gpsimd.ap_gather(tg[:, q0:q1, :], vbuf01[:, :, :],
                                idxw[:, q0 // 16:q1 // 16],
                                channels=128, num_elems=VCOLS, d=2, num_idxs=q1 - q0)

        def emit_marker(i, nblk_gate):
            # tiny STT reading the cat rows piece i will overwrite (WAR anchor)
            # and Mg[:, nblk_gate] (RAW anchor): pins catcopy/convT placement
            # to the energy loop's block count, immune to sim-clock drift.
            r0 = PB[i] * 2
            for ob in range(2):
                view = cats[2 + ob][:, :].rearrange(
                    "p (r wp) -> p r wp", wp=WP)[:, r0:r0 + 1, 1:W + 1]
                view = view.rearrange("p one w -> p (one w)")
                nc.vector.scalar_tensor_tensor(
                    mkscr[:, :], view, Mg[:, nblk_gate:nblk_gate + 1], view,
                    op0=ALU.mult, op1=ALU.add)

        def emit_catcopy(i):
            q0, q1 = PB[i] * 128, PB[i + 1] * 128
            tg = TGbox["tg"]
            r0, r1 = q0 // W, q1 // W
            for ob in range(2):
                src = tg[:, q0:q1, ob:ob + 1].rearrange("p q one -> p (q one)")
                src = src.rearrange("p (r w) -> p r w", w=W)
                dst = cats[2 + ob][:, :].rearrange(
                    "p (r wp) -> p r wp", wp=WP)[:, r0:r1, 1:W + 1]
                nc.scalar.copy(dst, src)

        def emit_convT(g):
            for ob in range(2):
                pc = PSB.tile([128, 512], F32, tag="ps512")
                emit_conv_half(ob, g, (2, 3), pc)
                nc.scalar.copy(tstage[:, ob * 4 + g, :], pc[:, :])

        # ---------- energy + argmax ----------
        piece = 0
        for b in range(NBLK):
            for c in range(NCHUNK):
                if b == 0:
                    k_chunk(2 * c)
                    k_chunk(2 * c + 1)
                pe = PSE.tile([128, 1024], F32, tag="pe")
                nc.tensor.matmul(pe[:, 0:512], qstack[:, b * 128:(b + 1) * 128],
                                 kstack[:, c * 1024:c * 1024 + 512], start=True, stop=True)
                nc.tensor.matmul(pe[:, 512:1024], qstack[:, b * 128:(b + 1) * 128],
                                 kstack[:, c * 1024 + 512:(c + 1) * 1024], start=True, stop=True)
                nc.vector.tensor_reduce(mch[:, c:c + 1], pe[:, :], axis=AX.X, op=ALU.max)
                nc.vector.scalar_tensor_tensor(
                    scr[:, :], pe[:, :], mch[:, c:c + 1],
                    iota16[:, c * 1024:(c + 1) * 1024],
                    op0=ALU.is_equal, op1=ALU.mult, accum_out=ach[:, c:c + 1])
            nc.vector.tensor_reduce(Mg[:, b:b + 1], mch[:, :], axis=AX.X, op=ALU.max)
            nc.vector.scalar_tensor_tensor(
                sel[:, :], mch[:, :], Mg[:, b:b + 1], ach[:, :],
                op0=ALU.is_equal, op1=ALU.mult, accum_out=Af[:, b:b + 1])
            if b == 0:
                for qc in range(1, 5):
                    q_chunk(qc)
            for f in fillers.get(b, []):
                f()
            if b == PB[1] - 1:
                # cx/fx/xh all consumed (q/k/v MMs emitted by block 4)
                ACT_cm.__exit__(None, None, None)
                TG_cm = tc.tile_pool(name="tgpool", bufs=1)
                TG = TG_cm.__enter__()
                tgtile = TG.tile([128, EXTQ, 2], F32, tag="tg")
                TGbox["tg"] = tgtile
                tstage = TG.tile([128, 8, 512], F32, tag="tstage")
            if b == PB[piece + 1] - 1:
                emit_arg_relayout(piece)
                emit_gather(piece)
                piece += 1
            if b in MARKER_AT:
                i = MARKER_AT[b]
                emit_marker(i, b)
                emit_catcopy(i)
                emit_convT(i)

        nc.sync.dma_start(T["dbg_s_d"].ap(), Mg[:, :])
        nc.sync.dma_start(T["dbg_arg_d"].ap(), Af[:, :])

        emit_catcopy(2)
        emit_convT(2)
        emit_catcopy(3)
        emit_convT(3)

        # S row: srow[q=b*128+p] = Mg[p,b]; then stride-0 broadcast of out cols
        ssrc = srow_t[:]
        sdst = AP(ssrc.tensor, ssrc.offset, [[1, 128], [128, NBLK]])
        nc.sync.dma_start(sdst, Mg[:, :])
        sbc = AP(ssrc.tensor, ssrc.offset + W, [[0, 128], [1, OUTP]])
        nc.sync.dma_start(s128[:, :], sbc)

        # ---------- assembly ----------
        S_cm = tc.tile_pool(name="stream", bufs=2)
        S = S_cm.__enter__()
        for g in range(4):
            for ob in range(2):
                stage = S.tile([128, 512], F32, tag="stage")
                nc.vector.scalar_tensor_tensor(
                    stage[:, :], tstage[:, ob * 4 + g, :], bfs[:, ob:ob + 1],
                    convacc[:, ob * 4 + g, :], op0=ALU.add, op1=ALU.add)
                nc.vector.tensor_tensor(stage[:, :], stage[:, :],
                                        s128[:, g * 512:(g + 1) * 512], op=ALU.mult)
                fcatv = cats[ob][:, :].rearrange("p (r wp) -> p r wp", wp=WP)
                front_mid = fcatv[:, g * 8 + 1:g * 8 + 9, 1:1 + W]
                nc.vector.tensor_tensor(stage[:, :], stage[:, :], front_mid, op=ALU.add)
                nc.sync.dma_start(T["out_d"].ap()[ob][:, g * 512:(g + 1) * 512],
                                  stage[:, :])
        S_cm.__exit__(None, None, None)
        TG_cm.__exit__(None, None, None)


def _prep_shared(inputs):
    """Weight prep shared by all cores: pre-transposed fp16 (+hi/lo for q/k)."""
    f16, f32 = np.float16, np.float32
    Wq, Wk, Wv = inputs["Wq"], inputs["Wk"], inputs["Wv"]
    Wf = inputs["Wf"].reshape(C, 2 * C, 9)

    wqkT = np.zeros((128, 8 * C8), f16)
    for which, Wx in ((0, Wq), (1, Wk)):
        for cb in range(2):
            blk = np.ascontiguousarray(Wx[:, cb * 128:(cb + 1) * 128].T)  # [128, 32] f32
            hi = blk.astype(f16)
            lo = (blk - hi.astype(f32)).astype(f16)
            base = ((which * 2 + cb) * 2) * C8
            wqkT[:, base:base + C8] = hi
            wqkT[:, base + C8:base + 2 * C8] = lo

    wvT = np.zeros((128, 4 * 128), f16)
    for ob in range(2):
        for cb in range(2):
            wvT[:, (cb * 2 + ob) * 128:(cb * 2 + ob + 1) * 128] = \
                Wv[ob * 128:(ob + 1) * 128, cb * 128:(cb + 1) * 128].T.astype(f16)

    wfT = np.zeros((128, 72 * 128), f16)
    for ob in range(2):
        for cb4 in range(4):
            for tap in range(9):
                col = ((cb4 * 9 + tap) * 2 + ob) * 128
                wfT[:, col:col + 128] = \
                    Wf[ob * 128:(ob + 1) * 128, cb4 * 128:(cb4 + 1) * 128, tap].T.astype(f16)

    iota16 = np.broadcast_to(np.arange(HWF, dtype=np.int16), (128, HWF)).copy()

    return {
        "wqkT": wqkT, "wvT": wvT, "wfT": wfT, "iota16": iota16,
        "bq": inputs["bq"].reshape(C8, 1).astype(f32),
        "bk": inputs["bk"].reshape(C8, 1).astype(f32),
        "bv": np.ascontiguousarray(inputs["bv"].reshape(2, 128).T).astype(f32),
        "bf": np.ascontiguousarray(inputs["bf"].reshape(2, 128).T).astype(f32),
    }


def _hilo(x):
    f16, f32 = np.float16, np.float32
    hi = x.astype(f16)
    lo = (x - hi.astype(f32)).astype(f16)
    return hi, lo


def _prep_core_inputs(inputs, shared, core):
    f16, f32 = np.float16, np.float32
    b, half = core // 2, core % 2
    r0 = half * RH

    def ext_rows(x):  # (C,H,W) -> (C,EXTR,W) with zero boundary row
        out = np.zeros((C, EXTR, W), x.dtype)
        lo, hi = r0 - 1, r0 + RH + 1
        slo, dlo = max(lo, 0), max(lo, 0) - lo
        shi = min(hi, H)
        out[:, dlo:dlo + shi - slo] = x[:, slo:shi]
        return out

    fx = inputs["front_x"][b].reshape(2, 128, HWF)
    fxh, fxl = _hilo(fx)
    cxe = ext_rows(inputs["cross_x"][b]).reshape(2, 128, EXTQ)
    cxh, cxl = _hilo(cxe)
    xh16 = inputs["front_x_hat"][b].reshape(2, 128, HWF).astype(f16)
    catf = np.zeros((C, EXTR, WP), f16)
    catf[:, :, 1:W + 1] = ext_rows(inputs["front_x"][b]).astype(f16)
    catf = catf.reshape(2, 128, CATW)

    valid = np.ones((EXTR, W), f32)
    if r0 == 0:
        valid[0] = 0.0
    if r0 + RH == H:
        valid[-1] = 0.0
    vq = valid.reshape(EXTQ)
    mask = np.empty((128, NBLK), f32)
    for blk in range(NBLK):
        mask[:, blk] = vq[blk * 128:(blk + 1) * 128]
    amask = (1.0 - mask) * HWF

    m = {
        "cxh": np.ascontiguousarray(cxh), "cxl": np.ascontiguousarray(cxl),
        "fxh": np.ascontiguousarray(fxh), "fxl": np.ascontiguousarray(fxl),
        "xh16": np.ascontiguousarray(xh16), "catf": np.ascontiguousarray(catf),
        "mask": mask, "amask": amask,
    }
    m.update(shared)
    return m


LAST_RES = None


def kernel(_trace=False, **inputs):
    global LAST_RES
    inputs = {k: np.asarray(v, dtype=np.float32) for k, v in inputs.items()}
    has_bqk = bool(np.any(inputs["bq"]) or np.any(inputs["bk"]))
    has_bv = bool(np.any(inputs["bv"]))
    nc = _build(has_bqk, has_bv)
    shared = _prep_shared(inputs)
    in_maps = [_prep_core_inputs(inputs, shared, core) for core in range(8)]
    kw = {"trace": True} if _trace else {}
    res = bass_utils.run_bass_kernel_spmd(nc, in_maps, core_ids=list(range(8)), **kw)
    LAST_RES = res
    out = np.empty((B, C, H, W), np.float32)
    for core in range(8):
        b, half = core // 2, core % 2
        o = res.results[core]["out"].reshape(C, RH, W)
        out[b, :, half * RH:(half + 1) * RH, :] = o
    return out


if __name__ == "__main__":
    rng = np.random.default_rng(0)
    ins = {
        "front_x": rng.standard_normal((B, C, H, W)).astype(np.float32),
        "cross_x": rng.standard_normal((B, C, H, W)).astype(np.float32),
        "front_x_hat": rng.standard_normal((B, C, H, W)).astype(np.float32),
        "Wq": (rng.standard_normal((C8, C)) / 16).astype(np.float32),
        "bq": np.zeros((C8,), np.float32),
        "Wk": (rng.standard_normal((C8, C)) / 16).astype(np.float32),
        "bk": np.zeros((C8,), np.float32),
        "Wv": (rng.standard_normal((C, C)) / 16).astype(np.float32),
        "bv": np.zeros((C,), np.float32),
        "Wf": (rng.standard_normal((C, 2 * C, 3, 3)) / 68).astype(np.float32),
        "bf": np.zeros((C,), np.float32),
    }
    out = kernel(**ins)
    print("kernel ran, out shape", out.shape, "std", out.std())
